# revision 1
# baseline (speedup 1.0000x reference)
"""MixHop layer (hop0 + A@h1 + A^2@h2) on 8 trn2 NeuronCores.

Strategy: 1D node partition (rows) across 8 cores, with a host-side global
row permutation that load-balances edges across cores and 128-row windows
(output is inverse-permuted on the host). Dense hop matmuls on TensorE.
SpMM = dma_gather of neighbor features (bf16, 512B rows, 4 SWDGE queues) +
one-hot scatter matmuls on TensorE accumulating into per-window PSUM tiles.
The one-hot-scaled stationary tile P_T[e, r] = val_e * (r == row_off_e) is
precomputed on the host (bf16) and streamed in with large DMAs. Cross-core
halo handled by two AllGathers (hcat=[h1|h2] bf16, g bf16).
"""
import heapq
import os
import sys

for p in ("/opt/trn_rl_repo", "/root/.axon_site/_ro/trn_rl_repo"):
    if os.path.isdir(p) and p not in sys.path:
        sys.path.append(p)

import numpy as np
import ml_dtypes

N = 50000
E = 600000
C = 128
CORES = 8
NW = 50                   # windows per core
RPC = NW * 128            # 6400 rows per core (padded)
NP = RPC * CORES          # 51200
_SIZES = [1, 1] + [2] * 24  # ramped supergroups (sum = 50)
GROUPS = []
_w = 0
for _s in _SIZES:
    GROUPS.append((_w, min(NW, _w + _s)))
    _w += _s
    if _w >= NW:
        break
SG = None
NQ = 4                    # SWDGE queues
GBUFS = 8                 # gather tile buffers per parity

TRACE = False
STAGES = int(os.environ.get("KM_STAGES", "5"))
PT_DVE = os.environ.get("KM_PT", "dma") == "dve"
_CACHE = {}


def _balance_perm(edge_row, edge_col):
    """Assign nodes to (core, window) slots balancing per-(slot, parity)
    edge counts. Returns perm[new_pos] = old_row ... actually returns
    relabel[old_row] = new_row, where new_row = core*RPC + window*128 + k.
    """
    # per-node degree by destination (row) and parity of... we balance the
    # ROW side: window load = sum over rows of deg(row) split by col parity.
    # Parity of col after relabel is unknown until relabel is fixed -> use
    # total degree for balancing (parities stay ~50/50 per window).
    deg = np.bincount(edge_row, minlength=N).astype(np.int64)
    order = np.argsort(-deg, kind="stable")  # high degree first
    nslots = CORES * NW
    # greedy: put next node into least-loaded (core,window) with space
    loads = [(0, s) for s in range(nslots)]
    heapq.heapify(loads)
    space = np.full(nslots, 128, np.int64)
    new_of_old = np.empty(NP, np.int64)
    fill_ptr = np.zeros(nslots, np.int64)
    for r in order:
        while True:
            load, s = heapq.heappop(loads)
            if space[s] > 0:
                break
        k = 128 - space[s]
        space[s] -= 1
        new_of_old[r] = s * 128 + k
        if space[s] > 0:
            heapq.heappush(loads, (load + deg[r], s))
    # pad nodes fill remaining slots
    rem = []
    for s in range(nslots):
        for k in range(128 - space[s], 128):
            rem.append(s * 128 + k)
    new_of_old[N:] = rem
    return new_of_old


def _build_plan(edge_row, edge_col, edge_val):
    relabel = _balance_perm(edge_row, edge_col)
    er = relabel[edge_row]
    ec = relabel[edge_col]

    core = er // RPC
    w = (er % RPC) // 128
    off = (er % 128).astype(np.int64)
    par = ((ec % 128) // 64).astype(np.int64)
    gidx = ((ec // 128) * 64 + (ec % 64)).astype(np.int16)

    gid = (core * NW + w) * 2 + par
    ngroups = CORES * NW * 2
    counts = np.bincount(gid, minlength=ngroups).reshape(CORES, NW, 2)
    Bw = np.maximum(1, ((counts.max(axis=0) + 127) // 128))  # [NW, 2]

    cstart = np.zeros((NW, 2), np.int64)
    calls = []
    cpos = 0
    for (w0, w1) in GROUPS:
        for p in (0, 1):
            ws = list(range(w0, w1))
            nch = int(Bw[w0:w1, p].sum())
            for wi in ws:
                cstart[wi, p] = cpos
                cpos += int(Bw[wi, p])
            calls.append(dict(par=p, ws=ws, cstart=cpos - nch, nch=nch))
    T = cpos

    order = np.argsort(gid, kind="stable")
    gs = np.zeros(ngroups + 1, np.int64)
    np.cumsum(counts.reshape(-1), out=gs[1:])
    rank = np.arange(E, dtype=np.int64) - gs[gid[order]]
    pos = cstart[w[order], par[order]] * 128 + rank
    flat = core[order] * (T * 128) + pos

    idx_p = np.zeros(CORES * T * 128, np.int16)
    idx_p[flat] = gidx[order]
    idx_p = idx_p.reshape(CORES, T, 128)

    pt = np.zeros((CORES * T * 128, 128), ml_dtypes.bfloat16)
    pt[flat, off[order]] = edge_val[order].astype(ml_dtypes.bfloat16)
    pt = pt.reshape(CORES, T, 128, 128).transpose(0, 2, 1, 3)
    pt = np.ascontiguousarray(pt.reshape(CORES, 128, T * 128))

    seg = idx_p.reshape(CORES, T * 128 // 16, 16)
    wrapped16 = seg.transpose(0, 2, 1)
    gidx_w = np.ascontiguousarray(np.tile(wrapped16, (1, 8, 1)))

    off_p = np.zeros(CORES * T * 128, np.float32)
    val_p = np.zeros(CORES * T * 128, np.float32)
    off_p[flat] = off[order].astype(np.float32)
    val_p[flat] = edge_val[order]
    off_tab = np.ascontiguousarray(
        off_p.reshape(CORES, T, 128).transpose(0, 2, 1))
    val_tab = np.ascontiguousarray(
        val_p.reshape(CORES, T, 128).transpose(0, 2, 1))
    return dict(Bw=Bw, cstart=cstart, calls=calls, T=T,
                pt=pt, gidx_w=gidx_w, relabel=relabel,
                off_tab=off_tab, val_tab=val_tab)


def _build_program(plan):
    import concourse.bass as bass
    import concourse.bacc as bacc
    import concourse.mybir as mybir
    import concourse.tile as tile

    dt = mybir.dt
    Bw, cstart, calls, T = plan["Bw"], plan["cstart"], plan["calls"], plan["T"]

    nc = bacc.Bacc("TRN2", target_bir_lowering=False, debug=False,
                   num_devices=CORES, num_swdge_queues=NQ)

    xT_d = nc.dram_tensor("xT", [128, RPC], dt.bfloat16, kind="ExternalInput")
    wb_d = nc.dram_tensor("wb", [128, 768], dt.bfloat16, kind="ExternalInput")
    pt_d = None
    if not PT_DVE:
        pt_d = nc.dram_tensor("ptt", [128, T * 128], dt.bfloat16, kind="ExternalInput")
    gix_d = nc.dram_tensor("gixt", [128, T * 8], dt.int16, kind="ExternalInput")
    if PT_DVE:
        off_d = nc.dram_tensor("offt", [128, T], dt.float32, kind="ExternalInput")
        val_d = nc.dram_tensor("valt", [128, T], dt.float32, kind="ExternalInput")
        iota_d = nc.dram_tensor("iota", [128, 128], dt.float32, kind="ExternalInput")
    out0_d = nc.dram_tensor("out0", [128, NW, 128], dt.float32, kind="ExternalOutput")
    out1_d = nc.dram_tensor("out1", [128, NW, 128], dt.float32, kind="ExternalOutput")
    out2_d = nc.dram_tensor("out2", [128, NW, 128], dt.float32, kind="ExternalOutput")

    qn = [0]

    with tile.TileContext(nc) as tc:
        with (
            tc.tile_pool(name="const", bufs=1) as constp,
            tc.tile_pool(name="gath", bufs=GBUFS) as gathp,
            tc.tile_pool(name="pt", bufs=2) as ptp,
            tc.tile_pool(name="ev", bufs=2) as evp,
            tc.tile_pool(name="psum", bufs=4, space="PSUM") as psp,
            tc.tile_pool(name="psd", bufs=4, space="PSUM") as psdp,
            tc.tile_pool(name="dram", bufs=1, space="DRAM") as dramp,
        ):
            xT = constp.tile([128, RPC], dt.bfloat16)
            nc.sync.dma_start(xT[:], xT_d[:])
            wb = constp.tile([128, 768], dt.bfloat16)
            nc.sync.dma_start(wb[:], wb_d[:])
            gixt = constp.tile([128, T * 8], dt.int16)
            nc.sync.dma_start(gixt[:], gix_d[:])
            ones = constp.tile([1, 128], dt.bfloat16)
            nc.vector.memset(ones[:], 1.0)
            if PT_DVE:
                offt = constp.tile([128, T], dt.float32)
                nc.sync.dma_start(offt[:], off_d[:])
                valt = constp.tile([128, T], dt.float32)
                nc.sync.dma_start(valt[:], val_d[:])
                iota = constp.tile([128, 128], dt.float32)
                nc.sync.dma_start(iota[:], iota_d[:])

            hcat_sh = [dramp.tile([RPC // 2, 256], dt.bfloat16, name=f"hsh{p}")
                       for p in (0, 1)]
            hcat_fl = [dramp.tile([NP // 2, 256], dt.bfloat16,
                                  addr_space="Shared", name=f"hfl{p}")
                       for p in (0, 1)]
            g_sh = [dramp.tile([RPC // 2, 128], dt.bfloat16, name=f"gsh{p}")
                    for p in (0, 1)]
            g_fl = [dramp.tile([NP // 2, 128], dt.bfloat16,
                               addr_space="Shared", name=f"gfl{p}")
                    for p in (0, 1)]

            # ---- dense phase, batched per DG windows ----
            DG = 5
            for w0 in range(0, NW, DG):
                nwg = min(DG, NW - w0)
                h0b = evp.tile([128, nwg, 128], dt.float32, tag="h0")
                h1b = evp.tile([128, nwg, 128], dt.bfloat16, tag="h1")
                h2b = evp.tile([128, nwg, 128], dt.bfloat16, tag="h2")
                for wl in range(nwg):
                    w = w0 + wl
                    ph = psdp.tile([128, 384], dt.float32, tag="ph")
                    nc.tensor.matmul(ph[:], ones[:], wb[0:1, 384:768],
                                     start=True, stop=False)
                    for j in range(3):
                        nc.tensor.matmul(ph[:, j * 128:(j + 1) * 128],
                                         xT[:, w * 128:(w + 1) * 128],
                                         wb[:, j * 128:(j + 1) * 128],
                                         start=False, stop=(j == 2))
                    nc.vector.tensor_copy(h0b[:, wl, :], ph[:, 0:128])
                    nc.vector.tensor_copy(h1b[:, wl, :], ph[:, 128:256])
                    nc.vector.tensor_copy(h2b[:, wl, :], ph[:, 256:384])
                nc.sync.dma_start(out0_d[:, w0:w0 + nwg, :], h0b[:])
                # node (w, p) -> parity p%2, local row w*64 + p//2
                # hcat row layout per node: [h1(128) | h2(128)]
                for par in (0, 1):
                    hv = hcat_sh[par][w0 * 64:(w0 + nwg) * 64, :].rearrange(
                        "(g a) (j c) -> a g j c", a=64, j=2)
                    nc.sync.dma_start(hv[:, :, 0, :],
                                      h1b[par * 64:(par + 1) * 64, :, :])
                    nc.sync.dma_start(hv[:, :, 1, :],
                                      h2b[par * 64:(par + 1) * 64, :, :])

            if STAGES >= 2:
                for par in (0, 1):
                    nc.gpsimd.collective_compute(
                        "AllGather", mybir.AluOpType.bypass,
                        replica_groups=[list(range(CORES))],
                        ins=[hcat_sh[par][:].opt()],
                        outs=[hcat_fl[par][:].opt()])

            def spmm_pass(src_fl, elem, out_cols, oud, evict_g):
                LAG = 3
                pend = {}

                def issue_gather(gi, p):
                    call = calls[gi * 2 + p]
                    nch = call["nch"]
                    cs = call["cstart"]
                    gt = gathp.tile([128, nch, elem], dt.bfloat16,
                                    tag=f"g{p}", name=f"gt{p}",
                                    bufs=6 if p == 0 else 5)
                    nc.gpsimd.dma_gather(
                        gt[:], src_fl[p][:, :],
                        gixt[:, cs * 8:(cs + nch) * 8],
                        num_idxs=nch * 128, num_idxs_reg=nch * 128,
                        elem_size=elem, elem_step=elem,
                        single_packet=False, queue_num=qn[0] % NQ)
                    qn[0] += 1
                    return (gt, cs)

                def _process_group(gi):
                    w0, w1 = GROUPS[gi]
                    nwg = w1 - w0
                    c0 = calls[gi * 2]["cstart"]
                    c1 = calls[gi * 2 + 1]["cstart"] + calls[gi * 2 + 1]["nch"]
                    gts = pend.pop(gi)
                    ptt = gts.pop("ptt")
                    ycb = evp.tile([128, nwg, 128], dt.float32, tag="yc",
                                   name="ycb")
                    gcb = None
                    if evict_g:
                        gcb = evp.tile([128, nwg, 128], dt.bfloat16, tag="gc",
                                       name="gcb")
                    for w in range(w0, w1):
                        nchw = int(Bw[w, 0] + Bw[w, 1])
                        ps = psp.tile([128, out_cols], dt.float32, tag="ps")
                        k = 0
                        for p in (0, 1):
                            gt, cs = gts[p]
                            for bch in range(int(Bw[w, p])):
                                cg = int(cstart[w, p]) + bch
                                lp = cg - cs
                                nc.tensor.matmul(
                                    ps[:],
                                    ptt[:, (cg - c0) * 128:(cg - c0 + 1) * 128],
                                    gt[:, lp, :],
                                    start=(k == 0), stop=(k == nchw - 1))
                                k += 1
                        nc.vector.tensor_copy(ycb[:, w - w0, :], ps[:, 0:128])
                        if evict_g:
                            nc.vector.tensor_copy(gcb[:, w - w0, :],
                                                  ps[:, 128:256])
                    nc.sync.dma_start(oud[:, w0:w1, :], ycb[:])
                    if evict_g:
                        for par in (0, 1):
                            gv = g_sh[par][w0 * 64:w1 * 64, :].rearrange(
                                "(g a) c -> a g c", a=64)
                            nc.scalar.dma_start(
                                gv[:], gcb[par * 64:(par + 1) * 64, :, :])

                nG = len(GROUPS)
                for gi in range(nG + LAG):
                    if gi < nG:
                        pend[gi] = {0: issue_gather(gi, 0)}
                    ok = gi - (LAG - 2)
                    if 0 <= ok < nG:
                        pend[ok][1] = issue_gather(ok, 1)
                        c0 = calls[ok * 2]["cstart"]
                        c1 = calls[ok * 2 + 1]["cstart"] +                             calls[ok * 2 + 1]["nch"]
                        ptt = ptp.tile([128, (c1 - c0) * 128], dt.bfloat16,
                                       tag="ptt", name="ptt", bufs=3)
                        if PT_DVE:
                            for cg in range(c0, c1):
                                nc.vector.tensor_scalar(
                                    ptt[:, (cg - c0) * 128:(cg - c0 + 1) * 128],
                                    iota[:], offt[:, cg:cg + 1],
                                    valt[:, cg:cg + 1],
                                    mybir.AluOpType.is_equal,
                                    mybir.AluOpType.mult)
                        else:
                            nc.scalar.dma_start(ptt[:],
                                                pt_d[:, c0 * 128:c1 * 128])
                        pend[ok]["ptt"] = ptt
                    pk = gi - LAG
                    if 0 <= pk < nG:
                        _process_group(pk)

            if STAGES >= 3:
                spmm_pass(hcat_fl, 256, 256, out1_d, True)

            if STAGES >= 4:
                for par in (0, 1):
                    nc.gpsimd.collective_compute(
                        "AllGather", mybir.AluOpType.bypass,
                        replica_groups=[list(range(CORES))],
                        ins=[g_sh[par][:].opt()],
                        outs=[g_fl[par][:].opt()])

            if STAGES >= 5:
                spmm_pass(g_fl, 128, 128, out2_d, False)

    nc.compile()
    return nc


def _prepare_inputs(x, W, b, plan):
    relabel = plan["relabel"]
    xpad = np.zeros((NP, C), np.float32)
    xpad[relabel[:N]] = x
    xT = xpad.T
    Wp = np.concatenate([W[0], W[1], W[2]], axis=1)
    biasrow = np.zeros((128, 384), np.float32)
    biasrow[0] = np.concatenate([b[0], b[1], b[2]])
    wb = np.concatenate([Wp, biasrow], axis=1)

    in_maps = []
    for c in range(CORES):
        in_maps.append({
            "xT": np.ascontiguousarray(xT[:, c * RPC:(c + 1) * RPC]).astype(ml_dtypes.bfloat16),
            "wb": wb.astype(ml_dtypes.bfloat16),
            "ptt": plan["pt"][c],
            "gixt": plan["gidx_w"][c],
        })
        if PT_DVE:
            in_maps[-1]["offt"] = plan["off_tab"][c]
            in_maps[-1]["valt"] = plan["val_tab"][c]
            in_maps[-1]["iota"] = np.broadcast_to(
                np.arange(128, dtype=np.float32), (128, 128)).copy()
            del in_maps[-1]["ptt"]
    return in_maps


def kernel(x, W, b, edge_val, edge_row, edge_col):
    x = np.asarray(x, np.float32)
    W = np.asarray(W, np.float32)
    b = np.asarray(b, np.float32)
    edge_val = np.asarray(edge_val, np.float32)
    edge_row = np.asarray(edge_row, np.int32)
    edge_col = np.asarray(edge_col, np.int32)

    from concourse.bass_utils import run_bass_kernel_spmd

    key = hash((edge_row.tobytes(), edge_col.tobytes(), edge_val.tobytes()))
    if key not in _CACHE:
        plan = _build_plan(edge_row, edge_col, edge_val)
        nc = _build_program(plan)
        _CACHE[key] = (plan, nc)
    plan, nc = _CACHE[key]

    in_maps = _prepare_inputs(x, W, b, plan)
    res = run_bass_kernel_spmd(nc, in_maps, core_ids=list(range(CORES)),
                               trace=TRACE)
    kernel.last_results = res
    parts = []
    for c in range(CORES):
        r = res.results[c]
        blk = np.stack([r["out0"], r["out1"], r["out2"]], axis=-2)
        # blk [128 p, NW, 3, 128c] -> rows (w,p): transpose to [NW, p, 3*128]
        parts.append(blk.transpose(1, 0, 2, 3).reshape(RPC, 384))
    full = np.concatenate(parts, axis=0)
    return np.ascontiguousarray(full[plan["relabel"][:N]])


if __name__ == "__main__":
    rng = np.random.default_rng(0)
    x = rng.standard_normal((N, C), dtype=np.float32)
    W = rng.standard_normal((3, C, C), dtype=np.float32) / np.sqrt(C)
    b = rng.standard_normal((3, C), dtype=np.float32) * 0.01
    ev = rng.random(E, dtype=np.float32)
    er = rng.integers(0, N, E, dtype=np.int32)
    ec = rng.integers(0, N, E, dtype=np.int32)
    out = kernel(x=x, W=W, b=b, edge_val=ev, edge_row=er, edge_col=ec)
    print(out.shape, out.dtype)



# revision 7
# speedup vs baseline: 1.9664x; 1.9664x over previous
"""MixHop layer (hop0 + A@h1 + A^2@h2) on 8 trn2 NeuronCores.

Strategy v2: 1D node partition (rows) across 8 cores with a host-side
load-balancing row permutation. Key algebraic restructuring:

    y1 = A_t @ (x W1 + b1) = (A_t @ x) W1 + s b1        s  = A_t 1
    y2 = A_t A_t (x W2 + b2) = (A_t A_t x) W2 + s2 b2   s2 = A_t s

so both SpMM passes operate on x directly. Pass 1 gathers rows of the
(replicated, input-staged) x table - no collective before it, and it can
start at t=0. Pass 2 gathers rows of Z = A_t@x, which is AllGathered in
window-chunks overlapped with pass-1 processing. The W1/W2 matmuls and
rank-1 bias updates are small per-window dense matmuls applied to the
transposed (PE-transpose) pass outputs. s and s2 are computed exactly on
the host from the edge list.

SpMM = dma_gather of 256B node rows (bf16, 4 SWDGE queues) + one-hot
scatter matmuls on TensorE accumulating per-window PSUM tiles. The
one-hot stationary P_T[e, r] = val_e * (r == row_off_e) is generated
on-chip by two batched DVE tensor_tensor ops per supergroup (broadcast
APs), or optionally streamed from HBM (KM_PT=dma). P_T, gather-index,
offset and value tables are shared by both passes.
"""
import heapq
import os
import sys

for p in ("/opt/trn_rl_repo", "/root/.axon_site/_ro/trn_rl_repo"):
    if os.path.isdir(p) and p not in sys.path:
        sys.path.append(p)

import numpy as np
import ml_dtypes

N = 50000
E = 600000
C = 128
CORES = 8
NW = 50                   # windows per core
RPC = NW * 128            # 6400 rows per core (padded)
NP = RPC * CORES          # 51200
_SIZES = [1, 1] + [2] * 24  # ramped supergroups (sum = 50)
GROUPS = []
_w = 0
for _s in _SIZES:
    GROUPS.append((_w, min(NW, _w + _s)))
    _w += _s
    if _w >= NW:
        break
# AG-Z window chunks; each chunk needs its own Shared output tensor
# (framework enforces a single writer per Shared tensor), so a single
# chunk = one AllGather per parity.
ZCHUNKS = [(0, 50)]
NQ = int(os.environ.get("KM_NQ", "4"))    # SWDGE queues
GBUFS = 8

TRACE = False
STAGES = int(os.environ.get("KM_STAGES", "4"))
PT_DVE = os.environ.get("KM_PT", "dve") == "dve"
_CACHE = {}


def _balance_perm(edge_row, edge_col):
    """Assign nodes to (core, window) slots balancing per-slot edge
    counts. Returns relabel[old_row] = new_row = core*RPC + window*128 + k.
    """
    deg = np.bincount(edge_row, minlength=N).astype(np.int64)
    order = np.argsort(-deg, kind="stable")  # high degree first
    nslots = CORES * NW
    loads = [(0, s) for s in range(nslots)]
    heapq.heapify(loads)
    space = np.full(nslots, 128, np.int64)
    new_of_old = np.empty(NP, np.int64)
    for r in order:
        while True:
            load, s = heapq.heappop(loads)
            if space[s] > 0:
                break
        k = 128 - space[s]
        space[s] -= 1
        new_of_old[r] = s * 128 + k
        if space[s] > 0:
            heapq.heappush(loads, (load + deg[r], s))
    rem = []
    for s in range(nslots):
        for k in range(128 - space[s], 128):
            rem.append(s * 128 + k)
    new_of_old[N:] = rem
    return new_of_old


def _wrap16(idx, T):
    """Pack flat element indices [CORES, T*128] into the SWDGE 16-lane
    wrapped + 8x replicated [CORES, 128, T*8] int16 format."""
    seg = idx.reshape(CORES, T * 128 // 16, 16)
    wrapped16 = seg.transpose(0, 2, 1)
    return np.ascontiguousarray(np.tile(wrapped16, (1, 8, 1)))


def _build_plan(edge_row, edge_col, edge_val):
    relabel = _balance_perm(edge_row, edge_col)
    er = relabel[edge_row]
    ec = relabel[edge_col]

    core = er // RPC
    w = (er % RPC) // 128
    off = (er % 128).astype(np.int64)
    par = ((ec % 128) // 64).astype(np.int64)

    # pass-1 gather index: row in the x table (per parity)
    gidx1 = ((ec // 128) * 64 + (ec % 64)).astype(np.int64)

    # pass-2 gather index: row in the chunked AG-Z table (per parity)
    wg = ec // 128                       # global source window
    core_s = wg // NW
    wl = wg % NW
    zbase = []
    chunk_of = np.empty(NW, np.int64)
    csz = np.empty(NW, np.int64)
    cw0 = np.empty(NW, np.int64)
    b = 0
    for (a0, a1) in ZCHUNKS:
        zbase.append(b)
        chunk_of[a0:a1] = len(zbase) - 1
        csz[a0:a1] = a1 - a0
        cw0[a0:a1] = a0
        b += CORES * 64 * (a1 - a0)
    zb = np.array(zbase)[chunk_of[wl]]
    gidx2 = (zb + core_s * 64 * csz[wl] + (wl - cw0[wl]) * 64
             + (ec % 64)).astype(np.int64)

    gid = (core * NW + w) * 2 + par
    ngroups = CORES * NW * 2
    counts = np.bincount(gid, minlength=ngroups).reshape(CORES, NW, 2)
    Bw = np.maximum(1, ((counts.max(axis=0) + 127) // 128))  # [NW, 2]

    cstart = np.zeros((NW, 2), np.int64)
    calls = []
    cpos = 0
    for (w0, w1) in GROUPS:
        for p in (0, 1):
            ws = list(range(w0, w1))
            nch = int(Bw[w0:w1, p].sum())
            for wi in ws:
                cstart[wi, p] = cpos
                cpos += int(Bw[wi, p])
            calls.append(dict(par=p, ws=ws, cstart=cpos - nch, nch=nch))
    T = cpos

    order = np.argsort(gid, kind="stable")
    gs = np.zeros(ngroups + 1, np.int64)
    np.cumsum(counts.reshape(-1), out=gs[1:])
    rank = np.arange(E, dtype=np.int64) - gs[gid[order]]
    pos = cstart[w[order], par[order]] * 128 + rank
    flat = core[order] * (T * 128) + pos

    idx1 = np.zeros(CORES * T * 128, np.int16)
    idx1[flat] = gidx1[order].astype(np.int16)
    idx2 = np.zeros(CORES * T * 128, np.int16)
    idx2[flat] = gidx2[order].astype(np.int16)
    gixt1 = _wrap16(idx1.reshape(CORES, T, 128), T)
    gixt2 = _wrap16(idx2.reshape(CORES, T, 128), T)

    off_p = np.zeros(CORES * T * 128, np.float32)
    val_p = np.zeros(CORES * T * 128, np.float32)
    off_p[flat] = off[order].astype(np.float32)
    val_p[flat] = edge_val[order]
    off_tab = np.ascontiguousarray(
        off_p.reshape(CORES, T, 128).transpose(0, 2, 1)).astype(ml_dtypes.bfloat16)
    val_tab = np.ascontiguousarray(
        val_p.reshape(CORES, T, 128).transpose(0, 2, 1)).astype(ml_dtypes.bfloat16)

    plan = dict(Bw=Bw, cstart=cstart, calls=calls, T=T,
                gixt1=gixt1, gixt2=gixt2, relabel=relabel,
                off_tab=off_tab, val_tab=val_tab)

    if not PT_DVE:
        pt = np.zeros((CORES * T * 128, 128), ml_dtypes.bfloat16)
        pt[flat, off[order]] = edge_val[order].astype(ml_dtypes.bfloat16)
        pt = pt.reshape(CORES, T, 128, 128).transpose(0, 2, 1, 3)
        plan["pt"] = np.ascontiguousarray(pt.reshape(CORES, 128, T * 128))

    # exact host-side row-sum vectors: s = A_t 1, s2 = A_t s
    s = np.zeros(NP, np.float64)
    np.add.at(s, er, edge_val.astype(np.float64))
    s2 = np.zeros(NP, np.float64)
    np.add.at(s2, er, edge_val.astype(np.float64) * s[ec])
    plan["s"] = s.astype(np.float32)
    plan["s2"] = s2.astype(np.float32)
    return plan


def _build_program(plan):
    import concourse.bass as bass
    import concourse.bacc as bacc
    import concourse.mybir as mybir
    import concourse.tile as tile

    dt = mybir.dt
    Bw, cstart, calls, T = plan["Bw"], plan["cstart"], plan["calls"], plan["T"]

    nc = bacc.Bacc("TRN2", target_bir_lowering=False, debug=False,
                   num_devices=CORES, num_swdge_queues=NQ)

    xT_d = nc.dram_tensor("xT", [128, RPC], dt.bfloat16, kind="ExternalInput")
    wb_d = nc.dram_tensor("wb", [128, 768], dt.bfloat16, kind="ExternalInput")
    idn_d = nc.dram_tensor("idn", [128, 128], dt.bfloat16, kind="ExternalInput")
    sv_d = nc.dram_tensor("sv", [1, 2 * RPC], dt.bfloat16, kind="ExternalInput")
    gx1_d = nc.dram_tensor("gx1", [128, T * 8], dt.int16, kind="ExternalInput")
    gx2_d = nc.dram_tensor("gx2", [128, T * 8], dt.int16, kind="ExternalInput")
    xt_d = [nc.dram_tensor(f"xt{p}", [NP // 2, 128], dt.bfloat16,
                           kind="ExternalInput") for p in (0, 1)]
    pt_d = None
    if not PT_DVE:
        pt_d = nc.dram_tensor("ptt", [128, T * 128], dt.bfloat16,
                              kind="ExternalInput")
    else:
        off_d = nc.dram_tensor("offt", [128, T], dt.bfloat16, kind="ExternalInput")
        val_d = nc.dram_tensor("valt", [128, T], dt.bfloat16, kind="ExternalInput")
        iota_d = nc.dram_tensor("iota", [128, 128], dt.bfloat16, kind="ExternalInput")
    out0_d = nc.dram_tensor("out0", [128, NW, 128], dt.bfloat16, kind="ExternalOutput")
    out1_d = nc.dram_tensor("out1", [128, NW, 128], dt.bfloat16, kind="ExternalOutput")
    out2_d = nc.dram_tensor("out2", [128, NW, 128], dt.bfloat16, kind="ExternalOutput")

    qn = [0]

    with tile.TileContext(nc) as tc:
        with (
            tc.tile_pool(name="const", bufs=1) as constp,
            tc.tile_pool(name="gath", bufs=GBUFS) as gathp,
            tc.tile_pool(name="pt", bufs=2) as ptp,
            tc.tile_pool(name="ev", bufs=2) as evp,
            tc.tile_pool(name="psum", bufs=3, space="PSUM") as psp,
            tc.tile_pool(name="pst", bufs=2, space="PSUM") as pstp,
            tc.tile_pool(name="psy", bufs=2, space="PSUM") as psyp,
            tc.tile_pool(name="psd", bufs=1, space="PSUM") as psdp,
            tc.tile_pool(name="dram", bufs=1, space="DRAM") as dramp,
        ):
            xT = constp.tile([128, RPC], dt.bfloat16)
            nc.sync.dma_start(xT[:], xT_d[:])
            wb = constp.tile([128, 768], dt.bfloat16)
            nc.sync.dma_start(wb[:], wb_d[:])
            idn = constp.tile([128, 128], dt.bfloat16)
            nc.sync.dma_start(idn[:], idn_d[:])
            sv = constp.tile([1, 2 * RPC], dt.bfloat16)
            nc.sync.dma_start(sv[:], sv_d[:])
            gixt1 = constp.tile([128, T * 8], dt.int16)
            nc.sync.dma_start(gixt1[:], gx1_d[:])
            gixt2 = constp.tile([128, T * 8], dt.int16)
            nc.sync.dma_start(gixt2[:], gx2_d[:])
            ones = constp.tile([1, 128], dt.bfloat16)
            nc.vector.memset(ones[:], 1.0)
            if PT_DVE:
                offt = constp.tile([128, T], dt.bfloat16)
                nc.sync.dma_start(offt[:], off_d[:])
                valt = constp.tile([128, T], dt.bfloat16)
                nc.sync.dma_start(valt[:], val_d[:])
                iota = constp.tile([128, 128], dt.bfloat16)
                nc.sync.dma_start(iota[:], iota_d[:])

            z_sh = [dramp.tile([RPC // 2, 128], dt.bfloat16, name=f"zsh{p}")
                    for p in (0, 1)]
            z_fl = [dramp.tile([NP // 2, 128], dt.bfloat16,
                               addr_space="Shared", name=f"zfl{p}")
                    for p in (0, 1)]

            # ---- dense hop-0 (runs concurrently with pass-1 spmm) ----
            DG = 5
            for w0 in range(0, NW, DG):
                nwg = min(DG, NW - w0)
                h0b = evp.tile([128, nwg, 128], dt.bfloat16, tag="h0")
                for wl in range(nwg):
                    w = w0 + wl
                    ph = psdp.tile([128, 128], dt.float32, tag="ph")
                    nc.tensor.matmul(ph[:], ones[:], wb[0:1, 384:512],
                                     start=True, stop=False)
                    nc.tensor.matmul(ph[:], xT[:, w * 128:(w + 1) * 128],
                                     wb[:, 0:128], start=False, stop=True)
                    nc.vector.tensor_copy(h0b[:, wl, :], ph[:])
                nc.sync.dma_start(out0_d[:, w0:w0 + nwg, :], h0b[:])

            def spmm_pass(src_fl, gixt, out_d, evict_z, wcol, bcol, soff,
                          ag_chunks):
                """One A_t@(.) pass + per-window transpose and dense
                W/bias post-matmul. wcol: W column block in wb; bcol: bias
                column block; soff: offset into sv for the rank-1 bias."""
                LAG = 3
                pend = {}
                agq = list(ag_chunks) if ag_chunks else []

                def issue_gather(gi, p):
                    call = calls[gi * 2 + p]
                    nch = call["nch"]
                    cs = call["cstart"]
                    gt = gathp.tile([128, nch, 128], dt.bfloat16,
                                    tag=f"g{p}", name=f"gt{p}",
                                    bufs=6 if p == 0 else 5)
                    nc.gpsimd.dma_gather(
                        gt[:], src_fl[p][:, :],
                        gixt[:, cs * 8:(cs + nch) * 8],
                        num_idxs=nch * 128, num_idxs_reg=nch * 128,
                        elem_size=128, elem_step=128,
                        single_packet=False, queue_num=qn[0] % NQ)
                    qn[0] += 1
                    return (gt, cs)

                def _process_group(gi):
                    w0, w1 = GROUPS[gi]
                    nwg = w1 - w0
                    c0 = calls[gi * 2]["cstart"]
                    gts = pend.pop(gi)
                    ptt = gts.pop("ptt")
                    zcb = evp.tile([128, nwg, 128], dt.bfloat16, tag="zc",
                                   name="zcb")
                    ztb = evp.tile([128, nwg, 128], dt.bfloat16, tag="zt",
                                   name="ztb")
                    ycb = evp.tile([128, nwg, 128], dt.bfloat16, tag="yc",
                                   name="ycb")
                    for w in range(w0, w1):
                        nchw = int(Bw[w, 0] + Bw[w, 1])
                        ps = psp.tile([128, 128], dt.float32, tag="ps")
                        k = 0
                        for p in (0, 1):
                            gt, cs = gts[p]
                            for bch in range(int(Bw[w, p])):
                                cg = int(cstart[w, p]) + bch
                                lp = cg - cs
                                nc.tensor.matmul(
                                    ps[:],
                                    ptt[:, (cg - c0) * 128:(cg - c0 + 1) * 128],
                                    gt[:, lp, :],
                                    start=(k == 0), stop=(k == nchw - 1))
                                k += 1
                        nc.vector.tensor_copy(zcb[:, w - w0, :], ps[:])
                    if evict_z:
                        for par in (0, 1):
                            zv = z_sh[par][w0 * 64:w1 * 64, :].rearrange(
                                "(g a) c -> a g c", a=64)
                            nc.scalar.dma_start(
                                zv[:], zcb[par * 64:(par + 1) * 64, :, :])
                    for w in range(w0, w1):
                        wl = w - w0
                        pst = pstp.tile([128, 128], dt.bfloat16, tag="pt")
                        nc.tensor.transpose(pst[:], zcb[:, wl, :], idn[:])
                        nc.vector.tensor_copy(ztb[:, wl, :], pst[:])
                    for w in range(w0, w1):
                        wl = w - w0
                        psy = psyp.tile([128, 128], dt.float32, tag="py")
                        nc.tensor.matmul(psy[:],
                                         sv[0:1, soff + w * 128:
                                            soff + (w + 1) * 128],
                                         wb[0:1, bcol:bcol + 128],
                                         start=True, stop=False)
                        nc.tensor.matmul(psy[:], ztb[:, wl, :],
                                         wb[:, wcol:wcol + 128],
                                         start=False, stop=True)
                        nc.vector.tensor_copy(ycb[:, wl, :], psy[:])
                    nc.sync.dma_start(out_d[:, w0:w1, :], ycb[:])
                    # issue AG-Z chunks whose windows are complete (with a
                    # few-group delay so the CC never stalls the gpsimd
                    # queue waiting on z_sh writes)
                    while agq and agq[0][1] + 4 <= w1:
                        a0, a1, zb = agq.pop(0)
                        for par in (0, 1):
                            nc.gpsimd.collective_compute(
                                "AllGather", mybir.AluOpType.bypass,
                                replica_groups=[list(range(CORES))],
                                ins=[z_sh[par][a0 * 64:a1 * 64, :].opt()],
                                outs=[z_fl[par][zb:zb + CORES * 64 * (a1 - a0),
                                                :].opt()])

                def _gen_ptt(gi):
                    c0 = calls[gi * 2]["cstart"]
                    c1 = calls[gi * 2 + 1]["cstart"] + calls[gi * 2 + 1]["nch"]
                    nch = c1 - c0
                    ptt = ptp.tile([128, nch, 128], dt.bfloat16,
                                   tag="ptt", name="ptt", bufs=3)
                    if PT_DVE:
                        off_b = offt[:, c0:c1].unsqueeze(2).broadcast_to(
                            (128, nch, 128))
                        val_b = valt[:, c0:c1].unsqueeze(2).broadcast_to(
                            (128, nch, 128))
                        iota_b = iota[:].unsqueeze(1).broadcast_to(
                            (128, nch, 128))
                        nc.vector.tensor_tensor(
                            ptt[:], iota_b, off_b, mybir.AluOpType.is_equal)
                        nc.vector.tensor_tensor(
                            ptt[:], ptt[:], val_b, mybir.AluOpType.mult)
                    else:
                        nc.scalar.dma_start(
                            ptt[:].rearrange("p c r -> p (c r)"),
                            pt_d[:, c0 * 128:c1 * 128])
                    # flatten view for process-time slicing
                    return ptt

                nG = len(GROUPS)
                for gi in range(nG + LAG):
                    if gi < nG:
                        pend[gi] = {0: issue_gather(gi, 0)}
                    ok = gi - (LAG - 2)
                    if 0 <= ok < nG:
                        pend[ok][1] = issue_gather(ok, 1)
                        pend[ok]["ptt"] = _gen_ptt(ok)[:].rearrange(
                            "p c r -> p (c r)")
                    pk = gi - LAG
                    if 0 <= pk < nG:
                        _process_group(pk)
                # flush any remaining AG-Z chunks
                while agq:
                    a0, a1, zb = agq.pop(0)
                    for par in (0, 1):
                        nc.gpsimd.collective_compute(
                            "AllGather", mybir.AluOpType.bypass,
                            replica_groups=[list(range(CORES))],
                            ins=[z_sh[par][a0 * 64:a1 * 64, :].opt()],
                            outs=[z_fl[par][zb:zb + CORES * 64 * (a1 - a0),
                                            :].opt()])

            zbase = []
            b = 0
            for (a0, a1) in ZCHUNKS:
                zbase.append((a0, a1, b))
                b += CORES * 64 * (a1 - a0)

            if STAGES >= 2:
                spmm_pass(xt_d, gixt1, out1_d, True, 128, 512, 0,
                          zbase if STAGES >= 3 else [])

            if STAGES >= 4:
                spmm_pass(z_fl, gixt2, out2_d, False, 256, 640, RPC, None)

    nc.compile()
    return nc


def _prepare_inputs(x, W, b, plan):
    relabel = plan["relabel"]
    xpad = np.zeros((NP, C), np.float32)
    xpad[relabel[:N]] = x
    xT = xpad.T
    Wp = np.concatenate([W[0], W[1], W[2]], axis=1)
    biasrow = np.zeros((128, 384), np.float32)
    biasrow[0] = np.concatenate([b[0], b[1], b[2]])
    wb = np.concatenate([Wp, biasrow], axis=1).astype(ml_dtypes.bfloat16)

    xq = xpad.astype(ml_dtypes.bfloat16)
    xp = np.ascontiguousarray(
        xq.reshape(CORES * NW, 2, 64, C).transpose(1, 0, 2, 3)
        .reshape(2, NP // 2, C))

    idn = np.eye(128, dtype=ml_dtypes.bfloat16)
    iota = np.broadcast_to(
        np.arange(128, dtype=np.float32), (128, 128)).astype(ml_dtypes.bfloat16)

    in_maps = []
    for c in range(CORES):
        sv = np.zeros((1, 2 * RPC), np.float32)
        sv[0, :RPC] = plan["s"][c * RPC:(c + 1) * RPC]
        sv[0, RPC:] = plan["s2"][c * RPC:(c + 1) * RPC]
        m = {
            "xT": np.ascontiguousarray(
                xT[:, c * RPC:(c + 1) * RPC]).astype(ml_dtypes.bfloat16),
            "wb": wb,
            "idn": idn,
            "sv": sv.astype(ml_dtypes.bfloat16),
            "gx1": plan["gixt1"][c],
            "gx2": plan["gixt2"][c],
            "xt0": xp[0],
            "xt1": xp[1],
        }
        if PT_DVE:
            m["offt"] = plan["off_tab"][c]
            m["valt"] = plan["val_tab"][c]
            m["iota"] = iota
        else:
            m["ptt"] = plan["pt"][c]
        in_maps.append(m)
    return in_maps


def kernel(x, W, b, edge_val, edge_row, edge_col):
    x = np.asarray(x, np.float32)
    W = np.asarray(W, np.float32)
    b = np.asarray(b, np.float32)
    edge_val = np.asarray(edge_val, np.float32)
    edge_row = np.asarray(edge_row, np.int32)
    edge_col = np.asarray(edge_col, np.int32)

    from concourse.bass_utils import run_bass_kernel_spmd

    key = hash((edge_row.tobytes(), edge_col.tobytes(), edge_val.tobytes()))
    if key not in _CACHE:
        plan = _build_plan(edge_row, edge_col, edge_val)
        nc = _build_program(plan)
        _CACHE[key] = (plan, nc)
    plan, nc = _CACHE[key]

    in_maps = _prepare_inputs(x, W, b, plan)
    res = run_bass_kernel_spmd(nc, in_maps, core_ids=list(range(CORES)),
                               trace=TRACE)
    kernel.last_results = res
    parts = []
    for c in range(CORES):
        r = res.results[c]
        blk = np.stack([np.asarray(r["out0"], np.float32),
                        np.asarray(r["out1"], np.float32),
                        np.asarray(r["out2"], np.float32)], axis=-2)
        # blk [128 k, NW, 3, 128c] -> rows (w,k): [NW, k, 3*128]
        parts.append(blk.transpose(1, 0, 2, 3).reshape(RPC, 384))
    full = np.concatenate(parts, axis=0)
    return np.ascontiguousarray(full[plan["relabel"][:N]])


if __name__ == "__main__":
    rng = np.random.default_rng(0)
    x = rng.standard_normal((N, C), dtype=np.float32)
    W = rng.standard_normal((3, C, C), dtype=np.float32) / np.sqrt(C)
    b = rng.standard_normal((3, C), dtype=np.float32) * 0.01
    ev = rng.random(E, dtype=np.float32)
    er = rng.integers(0, N, E, dtype=np.int32)
    ec = rng.integers(0, N, E, dtype=np.int32)
    out = kernel(x=x, W=W, b=b, edge_val=ev, edge_row=er, edge_col=ec)
    print(out.shape, out.dtype)
